# revision 28
# baseline (speedup 1.0000x reference)
"""Trainium2 Bass kernel: two-pass axial masked self-attention.

Reference computation (per pass, same weights both passes):
    qkv = x @ W + b ; q,k,v = split(qkv)
    S   = q @ k^T * C**-0.5            (per (batch, row) block of 256 tokens)
    S   = where(outer(mask,mask)==0, -1e4, S)
    y   = softmax(S) @ v
Pass 1 attends along M; pass 2 along N (inputs transposed), output swapped back.

Sharding: the B*N = 1024 independent attention blocks of pass 1 are split
across 8 NeuronCores (128 blocks each); pass 2 re-shards across B*M on the
host between the two launches of the same compiled program.

Device schedule (fp32 data, matmuls in float32r; blocks processed in pairs):
    host supplies xT [G, C, M] (feature-major) and packed mask columns
    qT = W_q^T stationary @ xT ; kT likewise   (pair-batched, moving dim 512)
    v  = xT stationary @ W_v                   (token-major)
    va = [v | 1 1]                             (ones cols -> row sums for free)
    ST[k,m] = kT.T @ qT                        (logits, transposed)
    G = exp(SCALE*ST + bias_k)                 bias_k = -1e4*(1-mask_k)
    [O | r r] = G.T @ va                       (r = softmax denominators)
    U = ones128.T @ va                         (column sums on all partitions)
    out[m] = (mask_q[m] ? O[m] : U) / (mask_q[m] ? r[m] : 256)
"""

import numpy as np
from contextlib import ExitStack

B, N, M, C = 4, 256, 256, 256
NCORES = 8
G = (B * N) // NCORES  # attention blocks per core per launch
SCALE = float(C) ** -0.5

_CACHE = {}


def _build(g_blocks, with_bias):
    import concourse.mybir as mybir
    import concourse.tile as tile
    from concourse import bacc
    from concourse.alu_op_type import AluOpType

    dt = mybir.dt
    f32 = dt.float32
    f32r = dt.float32r
    EXP = mybir.ActivationFunctionType.Exp
    CPY = mybir.ActivationFunctionType.Copy
    IDN = mybir.ActivationFunctionType.Identity

    assert g_blocks % 2 == 0
    nc = bacc.Bacc("TRN2", target_bir_lowering=False, debug=False)

    # xT: feature-major blocks, host-transposed
    XT = nc.dram_tensor("xt", [g_blocks, C, M], f32, kind="ExternalInput").ap()
    # packed per-block mask rows: 0=(m-1)*1e4, 1=(m-1)*1e4/SCALE, 2=1-m
    MP = nc.dram_tensor("mp", [g_blocks, 3, M], f32, kind="ExternalInput").ap()
    W = nc.dram_tensor("w", [C, 3 * C], f32, kind="ExternalInput").ap()
    BQ = nc.dram_tensor("b", [3 * C], f32, kind="ExternalInput").ap()
    Y = nc.dram_tensor("y", [g_blocks, M, C], f32, kind="ExternalOutput").ap()

    def r_(ap):
        return ap.bitcast(f32r)

    with tile.TileContext(nc) as tc, ExitStack() as ctx:
        pool = lambda name, bufs, **kw: ctx.enter_context(
            tc.tile_pool(name=name, bufs=bufs, **kw)
        )

        const = pool("const", 1)
        # weights: DMA to staging, rounding copy into f32r-consumed tiles
        w_sb = []
        for cc in range(2):
            stg = const.tile([128, 3 * C], f32, tag=f"wstg{cc}")
            nc.sync.dma_start(stg, W[cc * 128 : (cc + 1) * 128, :])
            t = const.tile([128, 3 * C], f32, tag=f"w{cc}")
            nc.gpsimd.tensor_copy(r_(t), stg)
            w_sb.append(t)
        bqk = []
        if with_bias:
            for fc in range(4):
                t = const.tile([128, 1], f32, tag=f"bqk{fc}")
                nc.sync.dma_start(t, BQ[fc * 128 : (fc + 1) * 128].unsqueeze(1))
                bqk.append(t)
        bv_row = const.tile([1, C], f32, tag="bvrow")
        nc.sync.dma_start(bv_row, BQ[2 * C : 3 * C].unsqueeze(0))
        bv_t = const.tile([128, C], f32, tag="bvt")
        nc.gpsimd.partition_broadcast(bv_t, bv_row)
        ones_f32 = const.tile([128, 128], f32, tag="ones32")
        nc.vector.memset(ones_f32, 1.0)
        ones_sq = const.tile([128, 128], f32, tag="ones")
        nc.vector.tensor_copy(r_(ones_sq), ones_f32)

        # persistent v-augmented tiles, ones columns written once
        NVA = 6
        p_vap = pool("vap", 1)
        va_ring = []
        for i in range(NVA):
            t = p_vap.tile([128, C + 2], f32, tag=f"va{i}")
            nc.vector.tensor_copy(r_(t[:, C : C + 2]), ones_f32[:, 0:2])
            va_ring.append(t)

        # SBUF pools
        p_xts = pool("xts", 2)
        p_xt = pool("xt", 2)
        p_q = pool("q", 3)
        p_k = pool("k", 3)
        p_g = pool("g", 3)
        p_mk = pool("mk", 4)
        p_bl = pool("bl", 6)
        p_out = pool("out", 3)

        # PSUM pools (8 banks)
        ps_qk = pool("ps_qk", 4, space="PSUM")
        ps_v = pool("ps_v", 1, space="PSUM")
        ps_st = pool("ps_st", 1, space="PSUM")
        ps_o = pool("ps_o", 2, space="PSUM")

        va_i = 0
        for gp in range(g_blocks // 2):
            # ---- load xT for the pair: [128, (cc, blk, m)] = 4 * 256 cols
            xt_stg = p_xts.tile([128, 4 * 256], f32, tag="xts")
            for cc in range(2):
                nc.sync.dma_start(
                    xt_stg[:, cc * 512 : (cc + 1) * 512].rearrange(
                        "p (blk m) -> p blk m", blk=2
                    ),
                    XT[2 * gp : 2 * gp + 2, cc * 128 : (cc + 1) * 128, :].rearrange(
                        "blk p m -> p blk m"
                    ),
                )
            xt = p_xt.tile([128, 4 * 256], f32, tag="xt")
            nc.scalar.activation(r_(xt), xt_stg, CPY)

            def xts(cc, blk):
                # xT chunk of one block: [128c, 256m]
                return xt[:, cc * 512 + blk * 256 : cc * 512 + (blk + 1) * 256]

            # ---- masks: key-bias cols for the pair, query rows per block
            mks, bqrows, invmqbs = [], [], []
            for blk in range(2):
                t = p_mk.tile([128, 2], f32, tag="mk")
                nc.sync.dma_start(
                    t, MP[2 * gp + blk, 0].rearrange("(h p) -> p h", p=128)
                )
                mks.append(t)
            for blk in range(2):
                rowstg = p_mk.tile([1, 2 * M], f32, tag="rowstg")
                nc.sync.dma_start(
                    rowstg.rearrange("o (two m) -> o two m", two=2),
                    MP[2 * gp + blk, 1:3].unsqueeze(0),
                )
                bqrow = p_mk.tile([1, M], f32, tag="bqrow")
                nc.vector.tensor_scalar_mul(r_(bqrow), rowstg[:, 0:M], 1.0)
                bqrows.append(bqrow)
                invmqb = p_mk.tile([128, M], f32, tag="invmqb")
                nc.gpsimd.partition_broadcast(invmqb, rowstg[:, M : 2 * M])
                invmqbs.append(invmqb)

            # ---- Q/K projections, pair-batched (moving dim 512)
            q_ps = [ps_qk.tile([128, 512], f32, tag="qk", name=f"qps{fc}") for fc in range(2)]
            k_ps = [ps_qk.tile([128, 512], f32, tag="qk", name=f"kps{fc}") for fc in range(2)]
            for fc in range(2):
                for cc in range(2):
                    nc.tensor.matmul(
                        q_ps[fc],
                        r_(w_sb[cc][:, fc * 128 : (fc + 1) * 128]),
                        r_(xt[:, cc * 512 : (cc + 1) * 512]),
                        start=(cc == 0),
                        stop=(cc == 1),
                    )
            for fc in range(2):
                for cc in range(2):
                    nc.tensor.matmul(
                        k_ps[fc],
                        r_(w_sb[cc][:, 256 + fc * 128 : 256 + (fc + 1) * 128]),
                        r_(xt[:, cc * 512 : (cc + 1) * 512]),
                        start=(cc == 0),
                        stop=(cc == 1),
                    )
            q_sb = [p_q.tile([128, 512], f32, tag="q", name=f"qsb{fc}") for fc in range(2)]
            k_sb = [p_k.tile([128, 512], f32, tag="k", name=f"ksb{fc}") for fc in range(2)]
            for fc in range(2):
                if with_bias:
                    nc.scalar.activation(r_(q_sb[fc]), q_ps[fc], IDN, bias=bqk[fc])
                    nc.scalar.activation(r_(k_sb[fc]), k_ps[fc], IDN, bias=bqk[2 + fc])
                else:
                    nc.scalar.activation(r_(q_sb[fc]), q_ps[fc], CPY)
                    nc.scalar.activation(r_(k_sb[fc]), k_ps[fc], CPY)

            def qs(cc, blk):
                # qT chunk of one block: [128c, 256m]
                return q_sb[cc][:, blk * 256 : (blk + 1) * 256]

            def ks(cc, blk, kt):
                # kT [128c, 128k] tile
                return k_sb[cc][:, blk * 256 + kt * 128 : blk * 256 + (kt + 1) * 128]

            # ---- per-block attention
            for blk in range(2):
                g = 2 * gp + blk
                mk = mks[blk]

                # v projection: [128t, (tc, f)]
                v_ps = ps_v.tile([128, 512], f32, tag="v")
                for tc_ in range(2):
                    for cc in range(2):
                        nc.tensor.matmul(
                            v_ps[:, tc_ * 256 : (tc_ + 1) * 256],
                            r_(xts(cc, blk)[:, tc_ * 128 : (tc_ + 1) * 128]),
                            r_(w_sb[cc][:, 2 * C : 3 * C]),
                            start=(cc == 0),
                            stop=(cc == 1),
                        )
                va = [va_ring[(va_i + kt) % NVA] for kt in range(2)]
                va_i += 2
                for kt in range(2):
                    nc.vector.tensor_tensor(
                        r_(va[kt][:, 0:C]),
                        v_ps[:, kt * 256 : (kt + 1) * 256],
                        bv_t,
                        AluOpType.add,
                    )

                # logits (transposed) + rank-1 query-mask bias, then exp
                st = ps_st.tile([128, 512], f32, tag="st")
                for kt in range(2):
                    for cc in range(2):
                        nc.tensor.matmul(
                            st[:, kt * 256 : (kt + 1) * 256],
                            r_(ks(cc, blk, kt)),
                            r_(qs(cc, blk)),
                            start=(cc == 0),
                            stop=False,
                        )
                    nc.tensor.matmul(
                        st[:, kt * 256 : (kt + 1) * 256],
                        r_(ones_sq[0:1, :]),
                        r_(bqrows[blk]),
                        start=False,
                        stop=True,
                    )
                g_sb = p_g.tile([128, 512], f32, tag="g")
                for kt in range(2):
                    nc.scalar.activation(
                        g_sb[:, kt * 256 : (kt + 1) * 256],
                        st[:, kt * 256 : (kt + 1) * 256],
                        EXP,
                        bias=mk[:, kt : kt + 1],
                        scale=SCALE,
                    )
                h_sb = p_g.tile([128, 512], f32, tag="h")
                nc.vector.tensor_tensor(
                    r_(h_sb).rearrange("p (a m) -> p a m", a=2),
                    g_sb.rearrange("p (a m) -> p a m", a=2),
                    invmqbs[blk].unsqueeze(1).broadcast_to((128, 2, M)),
                    AluOpType.add,
                )

                # [O | r r] = H.T @ va  (already blended), normalize, store
                out_sb = p_out.tile([128, 2 * C], f32, tag="out")
                for mt in range(2):
                    t_full = ps_o.tile([128, 512], f32, tag="o", name=f"ops{mt}")
                    t = t_full[:, 0 : C + 2]
                    for kt in range(2):
                        nc.tensor.matmul(
                            t,
                            r_(h_sb[:, kt * 256 + mt * 128 : kt * 256 + (mt + 1) * 128]),
                            r_(va[kt]),
                            start=(kt == 0),
                            stop=(kt == 1),
                        )
                    rinv = p_bl.tile([128, 1], f32, tag="rinv")
                    nc.vector.reciprocal(rinv, t[:, C : C + 1])
                    nc.vector.tensor_tensor(
                        out_sb[:, mt * 256 : (mt + 1) * 256],
                        t[:, 0:C],
                        rinv.broadcast_to((128, C)),
                        AluOpType.mult,
                    )
                nc.sync.dma_start(
                    Y[g].rearrange("(mt p) c -> p mt c", p=128),
                    out_sb.rearrange("p (mt c) -> p mt c", mt=2),
                )

    nc.compile()
    return nc


def _get_nc(g_blocks, with_bias=False):
    key = ("nc", g_blocks, with_bias)
    if key not in _CACHE:
        _CACHE[key] = _build(g_blocks, with_bias)
    return _CACHE[key]


def _launch(nc, xtb, mpb, W, b, trace=False):
    """xtb: [8g, C, M] feature-major blocks, mpb: [8g, 2, M] packed mask.
    Returns [8g, M, C] attention outputs (token-major)."""
    from concourse.bass_utils import run_bass_kernel_spmd

    g = xtb.shape[0] // NCORES
    in_maps = [
        {
            "xt": xtb[c * g : (c + 1) * g],
            "mp": mpb[c * g : (c + 1) * g],
            "w": W,
            "b": b,
        }
        for c in range(NCORES)
    ]
    res = run_bass_kernel_spmd(nc, in_maps, core_ids=list(range(NCORES)), trace=trace)
    out = np.concatenate([r["y"] for r in res.results], axis=0)
    return out, res


def _pack_mask(mb):
    # mb: [nblk, T] -> [nblk, 3, T]: key bias, scaled query bias, 1-mask
    bias = (mb - 1.0) * 1.0e4
    return np.ascontiguousarray(
        np.stack([bias, bias / SCALE, 1.0 - mb], axis=1).astype(np.float32)
    )


def kernel(x, mask, W_qkv, b_qkv):
    x = np.asarray(x, dtype=np.float32)
    mask = np.asarray(mask, dtype=np.float32)
    W = np.ascontiguousarray(np.asarray(W_qkv, dtype=np.float32))
    b = np.ascontiguousarray(np.asarray(b_qkv, dtype=np.float32))

    with_bias = bool(np.any(b != 0.0))
    nc = _get_nc(G, with_bias)

    # pass 1: attend along M; blocks are (b, n); xT layout [block, C, M]
    xt1 = np.ascontiguousarray(x.reshape(B * N, M, C).transpose(0, 2, 1))
    mp1 = _pack_mask(mask.reshape(B * N, M))
    y1, _ = _launch(nc, xt1, mp1, W, b)

    # pass 2: attend along N; blocks are (b, m); need y1 as [B*M, C, N]
    # y1: [(b,n), m, c] -> [b, m, c, n]
    xt2 = np.ascontiguousarray(y1.reshape(B, N, M, C).transpose(0, 2, 3, 1)).reshape(
        B * M, C, N
    )
    mp2 = _pack_mask(np.ascontiguousarray(mask.swapaxes(1, 2)).reshape(B * M, N))
    y2, _ = _launch(nc, xt2, mp2, W, b)
    out = y2.reshape(B, M, N, C).swapaxes(1, 2)
    return np.ascontiguousarray(out)
